# revision 1
# baseline (speedup 1.0000x reference)
"""Trainium2 Bass kernel for nn_CNNBackbone: conv1d(D->C,K=5) + BN + ReLU,
conv1d(C->C,K=5) + BN + ReLU, conv1d(C->D,1x1), masked mean over ragged lengths.

Strategy
--------
Data-parallel over batch: 32 samples -> 8 cores x 4 sample-slots.
Samples are sorted by (masked) length and assigned snake-style so each slot's
group of 8 (one per core) has near-uniform length; per-slot loop bounds are
compile-time constants derived from the group max (SPMD: one program, 8 cores).

Algebraic simplifications (host side):
 - BN folded into conv weights/biases.
 - The final 1x1 conv commutes with the masked sum:
     pooled = Wf @ (sum_{t<len} h2[:, t]) / len + bf        (len > 0)
   so h2 is reduced on-chip (fused into the conv2 ReLU epilogue via
   ScalarE accum_out; partially-masked chunks use a 0/1 mask multiply +
   reduce on DVE), and the 1x1 conv becomes a tiny per-sample fp32 matvec.
 - Computation is truncated past each slot-group's max length (rounded to 16).

Conv-as-matmul: channels on partitions, 5 taps = 5 PSUM-accumulated matmuls
with shifted rhs slices; bf16 operands, fp32 PSUM.
"""

import math

import numpy as np
import ml_dtypes

import concourse.bass as bass
import concourse.mybir as mybir
import concourse.tile as tile
from concourse import bacc
from concourse.bass_utils import run_bass_kernel_spmd

B, S, D, C, KW = 32, 2048, 128, 256, 5
P = 128
CH = 512            # full chunk (matmul free dim / PSUM bank)
GR = 16             # tail-chunk width granularity
NCORES = 8
NSLOTS = B // NCORES
CB = C // P         # channel blocks of 128
EPS = 1e-5
H0W = S + 4         # x^T buffer width (2 halo cols each side)
H1W = S + 8         # h1 buffer width
BF16 = ml_dtypes.bfloat16
F32 = mybir.dt.float32
BF = mybir.dt.bfloat16

_BUILD_CACHE: dict = {}
LAST_RESULTS = None  # BassKernelResults of the most recent run (for test harness)
TRACE = False        # set True (or env BASS_TRACE=1) to capture a profile


def _chunks(total):
    """Split `total` columns into 512-wide chunks plus a short tail."""
    ws = [CH] * (total // CH)
    if total % CH:
        ws.append(total % CH)
    return ws


def _build(slot_cfg):
    """Build + compile the SPMD Bass program.

    slot_cfg[j] = (L1, L2, c0): conv1/conv2 computed column counts (multiples
    of GR) and the count of fully-unmasked 512-chunks for slot j's group.
    """
    nc = bacc.Bacc(None, target_bir_lowering=False, debug=False)

    xT = nc.dram_tensor("xT", [NSLOTS, P, S], BF, kind="ExternalInput")
    msk = nc.dram_tensor("msk", [NSLOTS, S], BF, kind="ExternalInput")
    w1t = nc.dram_tensor("w1t", [P, KW, CB, P], BF, kind="ExternalInput")
    w2t = nc.dram_tensor("w2t", [P, KW, CB, CB, P], BF, kind="ExternalInput")
    wft = nc.dram_tensor("wft", [P, CB, P], F32, kind="ExternalInput")
    bias1 = nc.dram_tensor("bias1", [P, CB], F32, kind="ExternalInput")
    bias2 = nc.dram_tensor("bias2", [P, CB], F32, kind="ExternalInput")
    invl = nc.dram_tensor("invl", [P, NSLOTS], F32, kind="ExternalInput")
    bfe = nc.dram_tensor("bfe", [P, NSLOTS], F32, kind="ExternalInput")
    out = nc.dram_tensor("out", [P, NSLOTS], F32, kind="ExternalOutput")

    RELU = mybir.ActivationFunctionType.Relu
    ADD = mybir.AluOpType.add
    MUL = mybir.AluOpType.mult

    with tile.TileContext(nc) as tc:
        with (
            tc.tile_pool(name="consts", bufs=1) as consts,
            tc.tile_pool(name="h0p", bufs=2) as h0p,
            tc.tile_pool(name="h1p", bufs=2) as h1p,
            tc.tile_pool(name="mkp", bufs=2) as mkp,
            tc.tile_pool(name="scp", bufs=4) as scp,
            tc.tile_pool(name="psp", bufs=7, space="PSUM") as psp,
            tc.tile_pool(name="psv", bufs=1, space="PSUM") as psv,
        ):
            w1s = consts.tile([P, KW, CB, P], BF)
            w2s = consts.tile([P, KW, CB, CB, P], BF)
            wfs = consts.tile([P, CB, P], F32)
            b1s = consts.tile([P, CB], F32)
            b2s = consts.tile([P, CB], F32)
            invls = consts.tile([P, NSLOTS], F32)
            bfes = consts.tile([P, NSLOTS], F32)
            rowsums = consts.tile([P, NSLOTS, CB, S // CH + 1], F32)
            rs_red = consts.tile([P, NSLOTS, CB], F32)
            out_sb = consts.tile([P, NSLOTS], F32)

            h0_t = [None] * NSLOTS
            h1_t = [None] * NSLOTS
            mk_t = [None] * NSLOTS

            def emit_load(j, split_first=False):
                L1, L2, c0 = slot_cfg[j]
                if L1 == 0:
                    return
                h0 = h0p.tile([P, H0W], BF, tag="h0")
                h1 = h1p.tile([P, CB, H1W], BF, tag="h1")
                h0_t[j], h1_t[j] = h0, h1
                w = min(L1 + 2, S)
                if split_first:
                    # first chunk (+halo) on the scalar queue, remaining
                    # chunks as separate pieces on sync — each matmul chunk
                    # only waits for its own piece.
                    w0 = min(CH + 6, w)
                    nc.scalar.dma_start(h0[:, 2 : 2 + w0], xT[j, :, 0:w0])
                    a = w0
                    while a < w:
                        b_ = min(a + CH, w)
                        nc.sync.dma_start(h0[:, 2 + a : 2 + b_], xT[j, :, a:b_])
                        a = b_
                else:
                    nc.sync.dma_start(h0[:, 2 : 2 + w], xT[j, :, 0:w])
                nc.vector.memset(h0[:, 0:2], 0.0)
                if 2 + w < L1 + 4:
                    nc.vector.memset(h0[:, 2 + w : L1 + 4], 0.0)
                for cb in range(CB):
                    nc.vector.memset(h1[:, cb, 0:2], 0.0)
                    nc.vector.memset(h1[:, cb, 2 + L1 : 4 + L1], 0.0)
            def emit_mask(j):
                L1, L2, c0 = slot_cfg[j]
                wm = L2 - c0 * CH
                if L1 > 0 and wm > 0:
                    mk = mkp.tile([P, S], BF, tag="mk")
                    mk_t[j] = mk
                    src = msk[j, c0 * CH : c0 * CH + wm]
                    bsrc = bass.AP(
                        tensor=src.tensor, offset=src.offset,
                        ap=[[0, P]] + list(src.ap),
                    )
                    nc.gpsimd.dma_start(mk[:, 0:wm], bsrc)

            def emit_conv1(j):
                L1, L2, c0 = slot_cfg[j]
                h0, h1 = h0_t[j], h1_t[j]
                for c, wc in enumerate(_chunks(L1)):
                    for cb in range(CB):
                        ps = psp.tile([P, CH], F32, tag="ps")
                        for k in range(KW):
                            nc.tensor.matmul(
                                ps[:, 0:wc],
                                w1s[:, k, cb, :],
                                h0[:, c * CH + k : c * CH + k + wc],
                                start=(k == 0),
                                stop=(k == KW - 1),
                            )
                        nc.scalar.activation(
                            h1[:, cb, 2 + c * CH : 2 + c * CH + wc],
                            ps[:, 0:wc], RELU, bias=b1s[:, cb : cb + 1],
                        )

            def emit_conv2(j, full_last=False):
                L1, L2, c0 = slot_cfg[j]
                h1, mk = h1_t[j], mk_t[j]
                order = list(enumerate(_chunks(L2)))
                if full_last:
                    # masked chunks (long DVE epilogue chains) first, fully
                    # accumulated chunks (single fused ACT) last, so the
                    # kernel tail only waits on the short chain.
                    order = [cw for cw in order if cw[0] >= c0] + \
                            [cw for cw in order if cw[0] < c0]
                for c, wc in order:
                    for cb in range(CB):
                        ps = psp.tile([P, CH], F32, tag="ps")
                        idx = 0
                        for cib in range(CB):
                            for k in range(KW):
                                nc.tensor.matmul(
                                    ps[:, 0:wc],
                                    w2s[:, k, cib, cb, :],
                                    h1[:, cib, c * CH + k : c * CH + k + wc],
                                    start=(idx == 0),
                                    stop=(idx == CB * KW - 1),
                                )
                                idx += 1
                        col = rowsums[:, j, cb, c : c + 1]
                        h2 = scp.tile([P, CH], BF, tag="h2")
                        if c < c0:
                            # unmasked for every core in the group:
                            # ReLU + bias + rowsum fused on ScalarE
                            nc.scalar.activation(
                                h2[:, 0:wc], ps[:, 0:wc], RELU,
                                bias=b2s[:, cb : cb + 1], accum_out=col,
                            )
                        else:
                            nc.scalar.activation(
                                h2[:, 0:wc], ps[:, 0:wc], RELU,
                                bias=b2s[:, cb : cb + 1],
                            )
                            sc = scp.tile([P, CH], BF, tag="sc")
                            mslice = mk[:, (c - c0) * CH : (c - c0) * CH + wc]
                            nc.vector.tensor_tensor(
                                sc[:, 0:wc], h2[:, 0:wc], mslice, MUL,
                            )
                            nc.vector.tensor_reduce(
                                col, sc[:, 0:wc],
                                axis=mybir.AxisListType.X, op=ADD,
                            )

            def emit_slot_reduce(j):
                L1, L2, c0 = slot_cfg[j]
                n2c = len(_chunks(L2))
                for cb in range(CB):
                    if n2c == 0:
                        nc.vector.memset(rs_red[:, j, cb : cb + 1], 0.0)
                    elif n2c == 1:
                        nc.vector.tensor_copy(
                            rs_red[:, j, cb : cb + 1], rowsums[:, j, cb, 0:1]
                        )
                    else:
                        nc.vector.tensor_reduce(
                            rs_red[:, j, cb : cb + 1],
                            rowsums[:, j, cb, 0:n2c],
                            axis=mybir.AxisListType.X, op=ADD,
                        )
                # fold 1/len here so the kernel tail only does matvec + bias
                nc.vector.tensor_tensor(
                    rs_red[:, j, :], rs_red[:, j, :],
                    invls[:, j : j + 1].to_broadcast((P, CB)), MUL,
                )

            # ---- emission order ----
            # PE warmup: the first data DMAs cannot complete before ~4us of
            # per-partition descriptor processing, so spend that dead window
            # on dummy matmuls. 9 x 512 cols at the cold rate is ~3.8us of
            # sustained PE activity -- enough to flip the HAM clock gate to
            # 8/8 (2.4 GHz) before the first real matmul issues.
            warm_w = scp.tile([P, CH], BF, tag="warm")
            warm_ps = psp.tile([P, CH], F32, tag="ps")
            nc.gpsimd.memset(warm_w, 0.0)
            for _ in range(9):
                nc.tensor.matmul(warm_ps, warm_w[:, 0:P], warm_w,
                                 start=True, stop=True)

            # cb0 taps on the sync HWDGE queue (ahead of x pieces), cb1 taps
            # on gpsimd SWDGE; slot 0's first x chunk goes on the scalar
            # queue (behind only the ACT table load). The first 5 matmuls
            # (cb0, chunk 0) then have the earliest possible start.
            nc.sync.dma_start(w1s[:, :, 0, :], w1t[:, :, 0, :])
            nc.gpsimd.dma_start(w1s[:, :, 1, :], w1t[:, :, 1, :])
            emit_load(0, split_first=True)
            nc.gpsimd.dma_start(w2s, w2t[:])
            nc.scalar.dma_start(b1s, bias1[:])
            nc.scalar.dma_start(b2s, bias2[:])
            nc.scalar.dma_start(invls, invl[:])
            emit_load(1)
            emit_conv1(0)
            emit_mask(0)
            emit_mask(1)
            emit_load(2)
            emit_conv1(1)
            emit_conv2(0)
            emit_slot_reduce(0)
            emit_load(3)
            emit_mask(2)
            emit_mask(3)
            emit_conv1(2)
            emit_conv2(1)
            emit_slot_reduce(1)
            nc.scalar.dma_start(wfs, wft[:])
            nc.scalar.dma_start(bfes, bfe[:])
            # finals: 1x1-conv matvec (fp32) per sample; slots 0/1 are issued
            # before the last conv blocks so only the final slots' chain sits
            # on the kernel tail.
            pooled = psv.tile([P, NSLOTS], F32)

            def emit_matvec(j):
                for cb in range(CB):
                    nc.tensor.matmul(
                        pooled[:, j : j + 1],
                        wfs[:, cb, :],
                        rs_red[:, j, cb : cb + 1],
                        start=(cb == 0),
                        stop=(cb == CB - 1),
                    )

            emit_conv1(3)
            emit_conv2(3)
            emit_slot_reduce(3)
            emit_matvec(0)
            emit_matvec(1)
            emit_conv2(2, full_last=True)
            emit_slot_reduce(2)
            emit_matvec(3)
            emit_matvec(2)
            nc.vector.tensor_tensor(out_sb, pooled, bfes, ADD)
            nc.sync.dma_start(out[:], out_sb)

    nc.compile()
    return nc


def _prep(inputs):
    """Host-side: BN folding, weight packing, length-sorted slot assignment."""
    x = np.ascontiguousarray(np.asarray(inputs["x"], dtype=np.float32))
    spi = np.asarray(inputs["start_padding_indices"]).astype(np.int64).reshape(B)
    W1 = np.asarray(inputs["W1"], np.float32)
    b1 = np.asarray(inputs["b1"], np.float32)
    g1 = np.asarray(inputs["g1"], np.float32)
    be1 = np.asarray(inputs["be1"], np.float32)
    m1 = np.asarray(inputs["m1"], np.float32)
    v1 = np.asarray(inputs["v1"], np.float32)
    W2 = np.asarray(inputs["W2"], np.float32)
    b2 = np.asarray(inputs["b2"], np.float32)
    g2 = np.asarray(inputs["g2"], np.float32)
    be2 = np.asarray(inputs["be2"], np.float32)
    m2 = np.asarray(inputs["m2"], np.float32)
    v2 = np.asarray(inputs["v2"], np.float32)
    Wf = np.asarray(inputs["Wf"], np.float32)
    bf = np.asarray(inputs["bf"], np.float32)

    lens = np.where(spi == -1, S, spi)
    lens = np.clip(lens, 0, S).astype(np.int64)

    order = np.argsort(-lens, kind="stable")
    assign = order.reshape(NSLOTS, NCORES)  # [slot, core] -> sample idx

    slot_cfg = []
    for j in range(NSLOTS):
        lj = lens[assign[j]]
        lmax, lmin = int(lj.max()), int(lj.min())
        if lmax == 0:
            slot_cfg.append((0, 0, 0))
            continue
        L2 = min(math.ceil(lmax / GR) * GR, S)
        L1 = min(math.ceil(min(lmax + 2, S) / GR) * GR, S)
        c0 = min(lmin // CH, len(_chunks(L2)))
        slot_cfg.append((L1, L2, c0))
    slot_cfg = tuple(slot_cfg)

    # fold BN into conv weights/biases
    s1 = g1 / np.sqrt(v1 + EPS)
    W1f = W1 * s1[:, None, None]
    b1f = (b1 - m1) * s1 + be1
    s2 = g2 / np.sqrt(v2 + EPS)
    W2f = W2 * s2[:, None, None]
    b2f = (b2 - m2) * s2 + be2

    # pack weights: lhsT layouts (contraction channel on partitions)
    w1t = np.ascontiguousarray(
        W1f.reshape(CB, P, D, KW).transpose(2, 3, 0, 1)
    ).astype(BF16)  # [d, k, cb, co]
    w2t = np.ascontiguousarray(
        W2f.reshape(CB, P, CB, P, KW).transpose(3, 4, 2, 0, 1)
    ).astype(BF16)  # [ci, k, cib, cob, co]
    wft = np.ascontiguousarray(
        Wf[:, :, 0].reshape(D, CB, P).transpose(2, 1, 0)
    ).astype(np.float32)  # [ci, cib, d]
    bias1 = np.ascontiguousarray(b1f.reshape(CB, P).T).astype(np.float32)
    bias2 = np.ascontiguousarray(b2f.reshape(CB, P).T).astype(np.float32)

    t_idx = np.arange(S)
    in_maps = []
    for i in range(NCORES):
        xT_i = np.empty((NSLOTS, P, S), dtype=BF16)
        msk_i = np.zeros((NSLOTS, S), dtype=BF16)
        invl_i = np.empty((P, NSLOTS), dtype=np.float32)
        bfe_i = np.empty((P, NSLOTS), dtype=np.float32)
        for j in range(NSLOTS):
            b_idx = int(assign[j, i])
            L = int(lens[b_idx])
            xT_i[j] = x[b_idx].T.astype(BF16)
            msk_i[j] = (t_idx < L).astype(BF16)
            invl_i[:, j] = 1.0 / max(L, 1)
            bfe_i[:, j] = bf * (1.0 if L > 0 else 0.0)
        in_maps.append({
            "xT": xT_i, "msk": msk_i,
            "w1t": w1t, "w2t": w2t, "wft": wft,
            "bias1": bias1, "bias2": bias2,
            "invl": invl_i, "bfe": bfe_i,
        })
    return slot_cfg, assign, in_maps


def kernel(**inputs) -> np.ndarray:
    global LAST_RESULTS
    import os

    slot_cfg, assign, in_maps = _prep(inputs)
    nc = _BUILD_CACHE.get(slot_cfg)
    if nc is None:
        nc = _build(slot_cfg)
        _BUILD_CACHE[slot_cfg] = nc

    trace = TRACE or bool(os.environ.get("BASS_TRACE"))
    if trace:
        try:
            import antenv.axon_hooks  # noqa: F401  (absent in some containers)
        except ImportError:
            trace = False
    res = run_bass_kernel_spmd(
        nc, in_maps, core_ids=list(range(NCORES)), trace=trace,
    )
    LAST_RESULTS = res

    pooled = np.zeros((B, D), dtype=np.float32)
    for i in range(NCORES):
        out_i = np.asarray(res.results[i]["out"], dtype=np.float32)  # [P, NSLOTS]
        for j in range(NSLOTS):
            pooled[int(assign[j, i])] = out_i[:, j]
    return pooled



# revision 2
# speedup vs baseline: 1.2779x; 1.2779x over previous
"""Trainium2 Bass kernel for nn_CNNBackbone: conv1d(D->C,K=5) + BN + ReLU,
conv1d(C->C,K=5) + BN + ReLU, conv1d(C->D,1x1), masked mean over ragged lengths.

Strategy (v2: fp8 DoubleRow)
----------------------------
Data-parallel over batch: 32 samples -> 8 cores x 4 sample-slots, sorted by
length so each slot's group of 8 has near-uniform length; per-slot loop bounds
are compile-time constants from the group max (SPMD: one program, 8 cores).

Numerics: both convs run in fp8-e4m3 with MatmulPerfMode.DoubleRow (2 fp8
contraction elements per PE cell per cycle -> ~1.8x tensor throughput).
 - conv2 pairs the two 128-channel input blocks: h1 is stored [P, 2, T] fp8,
   which is exactly the DoubleRow rhs layout.
 - conv1 pairs adjacent taps; x is DMA'd twice into a [P, 2, W] tile with a
   one-column shift so tap pairs (0,1) and (2,3) are DoubleRow matmuls and
   tap 4 is a normal fp8 matmul.
 - weights are scaled by 16 (keeps e4m3 out of subnormals); the scale is
   folded downstream (h1 stored as 16*h1, rowsums folded via 1/(16L),
   1/(256L) host-side constants), so conv epilogues are a SINGLE fused
   scalar_tensor_tensor op: out = max(psum + 16*b, 0) with accum_out rowsum.
 - fp8 W2 quantization error is weight-correlated and survives pooling
   (~1.9e-2 rel); corrected by pooled += 0.5 * dW2eff @ pooled(h1), folded
   into one extra per-sample matvec with host-precomputed Wc = 0.5*Wf@dW2eff.
   Net rel err ~8e-3 (numpy-validated).

Engine split: PE matmuls; conv1 epilogue on DVE (fused stt), conv2 epilogue
on ScalarE ACT (bias+relu+accum); partial-length chunks use one fused
mask-multiply-accumulate stt on DVE. The final 1x1 conv commutes with the
masked mean so it is a tiny per-sample fp32 matvec.
"""

import math

import numpy as np
import ml_dtypes

import concourse.bass as bass
import concourse.mybir as mybir
import concourse.tile as tile
from concourse import bacc
from concourse.bass_utils import run_bass_kernel_spmd

B, S, D, C, KW = 32, 2048, 128, 256, 5
P = 128
CH = 512            # full chunk (matmul free dim / PSUM bank)
GR = 16             # tail-chunk width granularity
NCORES = 8
NSLOTS = B // NCORES
CB = C // P         # channel blocks of 128
EPS = 1e-5
WS = 16.0           # fp8 weight scale (power of 2)
H0W = S + 16        # x pair-buffer lane width (16-aligned)
H1W = S + 16        # h1 lane width (16-aligned)
NCH = S // CH + 1   # max chunks per slot
BF16 = ml_dtypes.bfloat16
NP8 = ml_dtypes.float8_e4m3
F32 = mybir.dt.float32
BF = mybir.dt.bfloat16
F8 = mybir.dt.float8e4
DR = mybir.MatmulPerfMode.DoubleRow

_BUILD_CACHE: dict = {}
LAST_RESULTS = None  # BassKernelResults of the most recent run (for test harness)
TRACE = False        # set True (or env BASS_TRACE=1) to capture a profile


def _chunks(total):
    """Split `total` columns into 512-wide chunks plus a short tail."""
    ws = [CH] * (total // CH)
    if total % CH:
        ws.append(total % CH)
    return ws


def _build(slot_cfg):
    """Build + compile the SPMD Bass program.

    slot_cfg[j] = (L1, L2, c0): conv1/conv2 computed column counts (multiples
    of GR) and the count of fully-unmasked 512-chunks for slot j's group.
    """
    nc = bacc.Bacc(None, target_bir_lowering=False, debug=False)

    xT = nc.dram_tensor("xT", [NSLOTS, P, S], F8, kind="ExternalInput")
    msk = nc.dram_tensor("msk", [NSLOTS, S], BF, kind="ExternalInput")
    w1t = nc.dram_tensor("w1t", [P, 2, 2, CB, P], F8, kind="ExternalInput")
    w14t = nc.dram_tensor("w14t", [P, CB, P], F8, kind="ExternalInput")
    w2t = nc.dram_tensor("w2t", [P, KW, CB, 2, P], F8, kind="ExternalInput")
    wft = nc.dram_tensor("wft", [P, CB, P], F32, kind="ExternalInput")
    wct = nc.dram_tensor("wct", [P, CB, P], F32, kind="ExternalInput")
    bias1 = nc.dram_tensor("bias1", [P, CB], F32, kind="ExternalInput")
    bias2 = nc.dram_tensor("bias2", [P, CB], F32, kind="ExternalInput")
    invl1 = nc.dram_tensor("invl1", [P, NSLOTS], F32, kind="ExternalInput")
    invl2 = nc.dram_tensor("invl2", [P, NSLOTS], F32, kind="ExternalInput")
    bfe = nc.dram_tensor("bfe", [P, NSLOTS], F32, kind="ExternalInput")
    out = nc.dram_tensor("out", [P, NSLOTS], F32, kind="ExternalOutput")

    RELU = mybir.ActivationFunctionType.Relu
    ADD = mybir.AluOpType.add
    MUL = mybir.AluOpType.mult
    MAX = mybir.AluOpType.max

    with tile.TileContext(nc) as tc:
        with (
            tc.tile_pool(name="consts", bufs=1) as consts,
            tc.tile_pool(name="h0p", bufs=2) as h0p,
            tc.tile_pool(name="h1p", bufs=2) as h1p,
            tc.tile_pool(name="mkp", bufs=2) as mkp,
            tc.tile_pool(name="scp", bufs=4) as scp,
            tc.tile_pool(name="psp", bufs=7, space="PSUM") as psp,
            tc.tile_pool(name="psv", bufs=1, space="PSUM") as psv,
        ):
            w1s = consts.tile([P, 2, 2, CB, P], F8)
            w14s = consts.tile([P, CB, P], F8)
            w2s = consts.tile([P, KW, CB, 2, P], F8)
            wfs = consts.tile([P, CB, P], F32)
            wcs = consts.tile([P, CB, P], F32)
            b1s = consts.tile([P, CB], F32)
            b2s = consts.tile([P, CB], F32)
            invl1s = consts.tile([P, NSLOTS], F32)
            invl2s = consts.tile([P, NSLOTS], F32)
            bfes = consts.tile([P, NSLOTS], F32)
            zcol = consts.tile([P, 1], F32)
            rs1 = consts.tile([P, NSLOTS, CB, NCH], F32)
            rs2 = consts.tile([P, NSLOTS, CB, NCH], F32)
            rs1_red = consts.tile([P, NSLOTS, CB], F32)
            rs2_red = consts.tile([P, NSLOTS, CB], F32)
            out_sb = consts.tile([P, NSLOTS], F32)

            h0_t = [None] * NSLOTS
            h1_t = [None] * NSLOTS
            mk_t = [None] * NSLOTS

            def emit_load(j, split_first=False):
                L1, L2, c0 = slot_cfg[j]
                if L1 == 0:
                    return
                h0 = h0p.tile([P, 2, H0W], F8, tag="h0")
                h1 = h1p.tile([P, CB, H1W], F8, tag="h1")
                h0_t[j], h1_t[j] = h0, h1
                w = min(L1 + 2, S)
                if split_first:
                    # first chunk (+halo) on the scalar queue, remaining
                    # chunks as separate pieces on sync — each matmul chunk
                    # only waits for its own piece.
                    w0 = min(CH + 6, w)
                    nc.scalar.dma_start(h0[:, 0, 2 : 2 + w0], xT[j, :, 0:w0])
                    nc.scalar.dma_start(h0[:, 1, 1 : 1 + w0], xT[j, :, 0:w0])
                    a = w0
                    while a < w:
                        b_ = min(a + CH, w)
                        nc.sync.dma_start(h0[:, 0, 2 + a : 2 + b_], xT[j, :, a:b_])
                        nc.sync.dma_start(h0[:, 1, 1 + a : 1 + b_], xT[j, :, a:b_])
                        a = b_
                else:
                    nc.sync.dma_start(h0[:, 0, 2 : 2 + w], xT[j, :, 0:w])
                    nc.sync.dma_start(h0[:, 1, 1 : 1 + w], xT[j, :, 0:w])
                nc.vector.memset(h0[:, 0, 0:2], 0.0)
                nc.vector.memset(h0[:, 1, 0:1], 0.0)
                if 2 + w < L1 + 4:
                    nc.vector.memset(h0[:, 0, 2 + w : L1 + 4], 0.0)
                if 1 + w < L1 + 2:
                    nc.vector.memset(h0[:, 1, 1 + w : L1 + 2], 0.0)
                for cb in range(CB):
                    nc.vector.memset(h1[:, cb, 0:2], 0.0)
                    nc.vector.memset(h1[:, cb, 2 + L1 : 4 + L1], 0.0)

            def emit_mask(j):
                L1, L2, c0 = slot_cfg[j]
                wm = L2 - c0 * CH
                if L1 > 0 and wm > 0:
                    mk = mkp.tile([P, S], BF, tag="mk")
                    mk_t[j] = mk
                    src = msk[j, c0 * CH : c0 * CH + wm]
                    bsrc = bass.AP(
                        tensor=src.tensor, offset=src.offset,
                        ap=[[0, P]] + list(src.ap),
                    )
                    nc.gpsimd.dma_start(mk[:, 0:wm], bsrc)

            def emit_conv1(j):
                L1, L2, c0 = slot_cfg[j]
                h0, h1 = h0_t[j], h1_t[j]
                for c, wc in enumerate(_chunks(L1)):
                    cs = c * CH
                    for cb in range(CB):
                        ps = psp.tile([P, CH], F32, tag="ps")
                        nc.tensor.matmul(
                            ps[:, 0:wc], w1s[:, 0, :, cb, :],
                            h0[:, :, cs : cs + wc],
                            start=True, stop=False, perf_mode=DR,
                        )
                        nc.tensor.matmul(
                            ps[:, 0:wc], w1s[:, 1, :, cb, :],
                            h0[:, :, cs + 2 : cs + 2 + wc],
                            start=False, stop=False, perf_mode=DR,
                        )
                        nc.tensor.matmul(
                            ps[:, 0:wc], w14s[:, cb, :],
                            h0[:, 0, cs + 4 : cs + 4 + wc],
                            start=False, stop=True,
                        )
                        # fused epilogue on DVE: h1 = max(psum + 16*b1, 0),
                        # rowsum accumulated for fully-unmasked chunks
                        col = rs1[:, j, cb, c : c + 1] if c < c0 else None
                        nc.vector.scalar_tensor_tensor(
                            h1[:, cb, 2 + cs : 2 + cs + wc],
                            ps[:, 0:wc], b1s[:, cb : cb + 1],
                            zcol.to_broadcast((P, wc)),
                            ADD, MAX, accum_out=col,
                        )

            def emit_rs1_partial(j):
                # masked h1 rowsums for partially-valid chunks: one fused
                # multiply+accumulate per (chunk, block) on DVE
                L1, L2, c0 = slot_cfg[j]
                h1, mk = h1_t[j], mk_t[j]
                for c, wc in enumerate(_chunks(L2)):
                    if c < c0:
                        continue
                    cs = c * CH
                    for cb in range(CB):
                        sc = scp.tile([P, CH], BF, tag="sc")
                        nc.vector.scalar_tensor_tensor(
                            sc[:, 0:wc],
                            h1[:, cb, 2 + cs : 2 + cs + wc], 1.0,
                            mk[:, cs - c0 * CH : cs - c0 * CH + wc],
                            MUL, MUL, accum_out=rs1[:, j, cb, c : c + 1],
                        )

            def emit_conv2(j, full_last=False):
                L1, L2, c0 = slot_cfg[j]
                h1, mk = h1_t[j], mk_t[j]
                order = list(enumerate(_chunks(L2)))
                if full_last:
                    # masked chunks (longer epilogue chains) first, fully
                    # accumulated chunks (single fused ACT) last, so the
                    # kernel tail only waits on the short chain.
                    order = [cw for cw in order if cw[0] >= c0] + \
                            [cw for cw in order if cw[0] < c0]
                for c, wc in order:
                    cs = c * CH
                    for cb in range(CB):
                        ps = psp.tile([P, CH], F32, tag="ps")
                        for k in range(KW):
                            nc.tensor.matmul(
                                ps[:, 0:wc],
                                w2s[:, k, cb, :, :],
                                h1[:, :, cs + k : cs + k + wc],
                                start=(k == 0), stop=(k == KW - 1),
                                perf_mode=DR,
                            )
                        h2 = scp.tile([P, CH], BF, tag="h2")
                        if c < c0:
                            # unmasked for every core in the group:
                            # ReLU + bias + rowsum fused on ScalarE
                            nc.scalar.activation(
                                h2[:, 0:wc], ps[:, 0:wc], RELU,
                                bias=b2s[:, cb : cb + 1],
                                accum_out=rs2[:, j, cb, c : c + 1],
                            )
                        else:
                            nc.scalar.activation(
                                h2[:, 0:wc], ps[:, 0:wc], RELU,
                                bias=b2s[:, cb : cb + 1],
                            )
                            sc = scp.tile([P, CH], BF, tag="sc")
                            nc.vector.scalar_tensor_tensor(
                                sc[:, 0:wc], h2[:, 0:wc], 1.0,
                                mk[:, cs - c0 * CH : cs - c0 * CH + wc],
                                MUL, MUL,
                                accum_out=rs2[:, j, cb, c : c + 1],
                            )

            def emit_slot_reduce(j):
                L1, L2, c0 = slot_cfg[j]
                n2c = len(_chunks(L2))
                for rs, red, invs in (
                    (rs2, rs2_red, invl2s),
                    (rs1, rs1_red, invl1s),
                ):
                    for cb in range(CB):
                        if n2c == 0:
                            nc.vector.memset(red[:, j, cb : cb + 1], 0.0)
                        elif n2c == 1:
                            nc.vector.tensor_copy(
                                red[:, j, cb : cb + 1], rs[:, j, cb, 0:1]
                            )
                        else:
                            nc.vector.tensor_reduce(
                                red[:, j, cb : cb + 1],
                                rs[:, j, cb, 0:n2c],
                                axis=mybir.AxisListType.X, op=ADD,
                            )
                    # fold 1/(scale*len) so the tail is only matvec + bias
                    nc.vector.tensor_tensor(
                        red[:, j, :], red[:, j, :],
                        invs[:, j : j + 1].to_broadcast((P, CB)), MUL,
                    )

            # ---- emission order ----
            # PE warmup: the first data DMAs cannot complete before ~1.5us of
            # descriptor processing after the ~7us engine preamble; dummy
            # matmuls keep the PE busy so the HAM clock gate is released
            # (8/8 = 2.4 GHz) by the time real matmuls issue.
            warm_w = scp.tile([P, CH], BF, tag="warm")
            warm_ps = psp.tile([P, CH], F32, tag="ps")
            nc.gpsimd.memset(warm_w, 0.0)
            nc.vector.memset(zcol, 0.0)
            for _ in range(6):
                nc.tensor.matmul(warm_ps, warm_w[:, 0:P], warm_w,
                                 start=True, stop=True)

            # conv1 weights on sync (ahead of x pieces); w2 on gpsimd SWDGE;
            # slot 0's first x chunk on the scalar queue (behind only the
            # ACT table load).
            nc.sync.dma_start(w1s, w1t[:])
            nc.sync.dma_start(w14s, w14t[:])
            emit_load(0, split_first=True)
            nc.gpsimd.dma_start(w2s, w2t[:])
            nc.scalar.dma_start(b1s, bias1[:])
            nc.scalar.dma_start(b2s, bias2[:])
            nc.scalar.dma_start(invl1s, invl1[:])
            nc.scalar.dma_start(invl2s, invl2[:])
            emit_load(1)
            emit_mask(0)
            emit_conv1(0)
            emit_mask(1)
            emit_load(2)
            emit_conv1(1)
            emit_conv2(0)
            emit_rs1_partial(0)
            emit_slot_reduce(0)
            emit_load(3)
            emit_mask(2)
            emit_mask(3)
            emit_conv1(2)
            emit_conv2(1)
            emit_rs1_partial(1)
            emit_slot_reduce(1)
            nc.scalar.dma_start(wfs, wft[:])
            nc.scalar.dma_start(wcs, wct[:])
            nc.scalar.dma_start(bfes, bfe[:])
            # finals: 1x1-conv matvec + fp8-correction matvec (fp32) per
            # sample; slots 0/1 are issued before the last conv blocks so
            # only the final slots' chain sits on the kernel tail.
            pooled = psv.tile([P, NSLOTS], F32)

            def emit_matvec(j):
                ops = [(wfs, rs2_red), (wcs, rs1_red)]
                n = 2 * CB
                i = 0
                for w_, r_ in ops:
                    for cb in range(CB):
                        nc.tensor.matmul(
                            pooled[:, j : j + 1],
                            w_[:, cb, :],
                            r_[:, j, cb : cb + 1],
                            start=(i == 0),
                            stop=(i == n - 1),
                        )
                        i += 1

            emit_conv1(3)
            emit_conv2(3)
            emit_rs1_partial(3)
            emit_slot_reduce(3)
            emit_matvec(0)
            emit_matvec(1)
            emit_conv2(2, full_last=True)
            emit_rs1_partial(2)
            emit_slot_reduce(2)
            emit_matvec(3)
            emit_matvec(2)
            nc.vector.tensor_tensor(out_sb, pooled, bfes, ADD)
            nc.sync.dma_start(out[:], out_sb)

    nc.compile()
    return nc


def _prep(inputs):
    """Host-side: BN folding, fp8 weight packing, length-sorted slots."""
    x = np.ascontiguousarray(np.asarray(inputs["x"], dtype=np.float32))
    spi = np.asarray(inputs["start_padding_indices"]).astype(np.int64).reshape(B)
    W1 = np.asarray(inputs["W1"], np.float32)
    b1 = np.asarray(inputs["b1"], np.float32)
    g1 = np.asarray(inputs["g1"], np.float32)
    be1 = np.asarray(inputs["be1"], np.float32)
    m1 = np.asarray(inputs["m1"], np.float32)
    v1 = np.asarray(inputs["v1"], np.float32)
    W2 = np.asarray(inputs["W2"], np.float32)
    b2 = np.asarray(inputs["b2"], np.float32)
    g2 = np.asarray(inputs["g2"], np.float32)
    be2 = np.asarray(inputs["be2"], np.float32)
    m2 = np.asarray(inputs["m2"], np.float32)
    v2 = np.asarray(inputs["v2"], np.float32)
    Wf = np.asarray(inputs["Wf"], np.float32)
    bf = np.asarray(inputs["bf"], np.float32)

    lens = np.where(spi == -1, S, spi)
    lens = np.clip(lens, 0, S).astype(np.int64)

    order = np.argsort(-lens, kind="stable")
    assign = order.reshape(NSLOTS, NCORES)  # [slot, core] -> sample idx

    slot_cfg = []
    for j in range(NSLOTS):
        lj = lens[assign[j]]
        lmax, lmin = int(lj.max()), int(lj.min())
        if lmax == 0:
            slot_cfg.append((0, 0, 0))
            continue
        L2 = min(math.ceil(lmax / GR) * GR, S)
        L1 = min(math.ceil(min(lmax + 2, S) / GR) * GR, S)
        c0 = min(lmin // CH, len(_chunks(L2)))
        slot_cfg.append((L1, L2, c0))
    slot_cfg = tuple(slot_cfg)

    # fold BN into conv weights/biases
    s1 = g1 / np.sqrt(v1 + EPS)
    W1f = W1 * s1[:, None, None]
    b1f = (b1 - m1) * s1 + be1
    s2 = g2 / np.sqrt(v2 + EPS)
    W2f = W2 * s2[:, None, None]
    b2f = (b2 - m2) * s2 + be2

    # fp8 weights, scaled by WS (scale folded downstream)
    W1q = np.clip(W1f * WS, -240, 240).astype(NP8)   # [C, D, K]
    W2q = np.clip(W2f * WS, -240, 240).astype(NP8)   # [C, C, K]

    # conv1 DoubleRow packs: [d, pair, i, cb, co] for taps 0..3, tap 4 alone
    a1 = np.asarray(W1q).reshape(CB, P, D, KW)                 # [cb, co, d, k]
    w1t = np.ascontiguousarray(
        a1[:, :, :, 0:4].reshape(CB, P, D, 2, 2).transpose(2, 3, 4, 0, 1)
    )                                                          # [d, p, i, cb, co]
    w14t = np.ascontiguousarray(a1[:, :, :, 4].transpose(2, 0, 1))  # [d, cb, co]
    # conv2 DoubleRow pack: pair = input channel block
    a2 = np.asarray(W2q).reshape(CB, P, 2, P, KW)              # [cob, co, i, ci, k]
    w2t = np.ascontiguousarray(a2.transpose(3, 4, 0, 2, 1))    # [ci, k, cob, i, co]

    # fp8 W2 error correction: Wc = 0.5 * Wf @ sum_k(W2f - deq(W2q))
    dW2eff = (W2f - np.asarray(W2q).astype(np.float32) / WS).sum(axis=2)  # [co, ci]
    Wc = 0.5 * (Wf[:, :, 0] @ dW2eff)                          # [d, ci]

    wft = np.ascontiguousarray(
        Wf[:, :, 0].reshape(D, CB, P).transpose(2, 1, 0)
    ).astype(np.float32)  # [ci, cb, d]
    wct = np.ascontiguousarray(
        Wc.reshape(D, CB, P).transpose(2, 1, 0)
    ).astype(np.float32)
    bias1 = np.ascontiguousarray((WS * b1f).reshape(CB, P).T).astype(np.float32)
    bias2 = np.ascontiguousarray((WS * WS * b2f).reshape(CB, P).T).astype(np.float32)

    t_idx = np.arange(S)
    in_maps = []
    for i in range(NCORES):
        xT_i = np.empty((NSLOTS, P, S), dtype=NP8)
        msk_i = np.zeros((NSLOTS, S), dtype=BF16)
        invl1_i = np.empty((P, NSLOTS), dtype=np.float32)
        invl2_i = np.empty((P, NSLOTS), dtype=np.float32)
        bfe_i = np.empty((P, NSLOTS), dtype=np.float32)
        for j in range(NSLOTS):
            b_idx = int(assign[j, i])
            L = int(lens[b_idx])
            xT_i[j] = np.clip(x[b_idx].T, -240, 240).astype(NP8)
            msk_i[j] = (t_idx < L).astype(BF16)
            invl1_i[:, j] = 1.0 / (WS * max(L, 1))
            invl2_i[:, j] = 1.0 / (WS * WS * max(L, 1))
            bfe_i[:, j] = bf * (1.0 if L > 0 else 0.0)
        in_maps.append({
            "xT": xT_i, "msk": msk_i,
            "w1t": w1t, "w14t": w14t, "w2t": w2t,
            "wft": wft, "wct": wct,
            "bias1": bias1, "bias2": bias2,
            "invl1": invl1_i, "invl2": invl2_i, "bfe": bfe_i,
        })
    return slot_cfg, assign, in_maps


def kernel(**inputs) -> np.ndarray:
    global LAST_RESULTS
    import os

    slot_cfg, assign, in_maps = _prep(inputs)
    nc = _BUILD_CACHE.get(slot_cfg)
    if nc is None:
        nc = _build(slot_cfg)
        _BUILD_CACHE[slot_cfg] = nc

    trace = TRACE or bool(os.environ.get("BASS_TRACE"))
    if trace:
        try:
            import antenv.axon_hooks  # noqa: F401  (absent in some containers)
        except ImportError:
            trace = False
    res = run_bass_kernel_spmd(
        nc, in_maps, core_ids=list(range(NCORES)), trace=trace,
    )
    LAST_RESULTS = res

    pooled = np.zeros((B, D), dtype=np.float32)
    for i in range(NCORES):
        out_i = np.asarray(res.results[i]["out"], dtype=np.float32)  # [P, NSLOTS]
        for j in range(NSLOTS):
            pooled[int(assign[j, i])] = out_i[:, j]
    return pooled


# revision 16
# speedup vs baseline: 1.4644x; 1.1459x over previous
"""Trainium2 Bass kernel for nn_CNNBackbone: conv1d(D->C,K=5) + BN + ReLU,
conv1d(C->C,K=5) + BN + ReLU, conv1d(C->D,1x1), masked mean over ragged lengths.

Strategy (v2: fp8 DoubleRow)
----------------------------
Data-parallel over batch: 32 samples -> 8 cores x 4 sample-slots, sorted by
length so each slot's group of 8 has near-uniform length; per-slot loop bounds
are compile-time constants from the group max (SPMD: one program, 8 cores).

Numerics: both convs run in fp8-e4m3 with MatmulPerfMode.DoubleRow (2 fp8
contraction elements per PE cell per cycle -> ~1.8x tensor throughput).
 - conv2 pairs the two 128-channel input blocks: h1 is stored [P, 2, T] fp8,
   which is exactly the DoubleRow rhs layout.
 - conv1 pairs adjacent taps; x is DMA'd twice into a [P, 2, W] tile with a
   one-column shift so tap pairs (0,1) and (2,3) are DoubleRow matmuls and
   tap 4 is a normal fp8 matmul.
 - weights are scaled by 16 (keeps e4m3 out of subnormals); the scale is
   folded downstream (h1 stored as 16*h1, rowsums folded via 1/(16L),
   1/(256L) host-side constants), so conv epilogues are a SINGLE fused
   scalar_tensor_tensor op: out = max(psum + 16*b, 0) with accum_out rowsum.
 - fp8 W2 quantization error is weight-correlated and survives pooling
   (~1.9e-2 rel); corrected by pooled += 0.5 * dW2eff @ pooled(h1), folded
   into one extra per-sample matvec with host-precomputed Wc = 0.5*Wf@dW2eff.
   Net rel err ~8e-3 (numpy-validated).

Engine split: PE matmuls; conv1 epilogue on DVE (fused stt), conv2 epilogue
on ScalarE ACT (bias+relu+accum); partial-length chunks use one fused
mask-multiply-accumulate stt on DVE. The final 1x1 conv commutes with the
masked mean so it is a tiny per-sample fp32 matvec.
"""

import math

import numpy as np
import ml_dtypes

import concourse.bass as bass
import concourse.mybir as mybir
import concourse.tile as tile
from concourse import bacc
from concourse.bass_utils import run_bass_kernel_spmd

B, S, D, C, KW = 32, 2048, 128, 256, 5
P = 128
CH = 512            # full chunk (matmul free dim / PSUM bank)
GR = 16             # tail-chunk width granularity
NCORES = 8
NSLOTS = B // NCORES
CB = C // P         # channel blocks of 128
EPS = 1e-5
WS = 16.0           # fp8 weight scale (power of 2)
H0W = S + 16        # x pair-buffer lane width (16-aligned)
H1W = S + 16        # h1 lane width (16-aligned)
NCH = S // CH + 1   # max chunks per slot
BF16 = ml_dtypes.bfloat16
NP8 = ml_dtypes.float8_e4m3
F32 = mybir.dt.float32
BF = mybir.dt.bfloat16
F8 = mybir.dt.float8e4
DR = mybir.MatmulPerfMode.DoubleRow

_BUILD_CACHE: dict = {}
LAST_RESULTS = None  # BassKernelResults of the most recent run (for test harness)
TRACE = False        # set True (or env BASS_TRACE=1) to capture a profile


def _chunks(total):
    """Split `total` columns into 512-wide chunks plus a short tail."""
    ws = [CH] * (total // CH)
    if total % CH:
        ws.append(total % CH)
    return ws


def _build(slot_cfg):
    """Build + compile the SPMD Bass program.

    slot_cfg[j] = (L1, L2, c0): conv1/conv2 computed column counts (multiples
    of GR) and the count of fully-unmasked 512-chunks for slot j's group.
    """
    nc = bacc.Bacc(None, target_bir_lowering=False, debug=False)

    xT = nc.dram_tensor("xT", [NSLOTS, P, S], F8, kind="ExternalInput")
    msk = nc.dram_tensor("msk", [NSLOTS, S], BF, kind="ExternalInput")
    w1t = nc.dram_tensor("w1t", [P, 2, 2, CB, P], F8, kind="ExternalInput")
    w14t = nc.dram_tensor("w14t", [P, CB, P], F8, kind="ExternalInput")
    w2t = nc.dram_tensor("w2t", [P, KW, CB, 2, P], F8, kind="ExternalInput")
    wft = nc.dram_tensor("wft", [P, CB, P], BF, kind="ExternalInput")
    wct = nc.dram_tensor("wct", [P, CB, P], BF, kind="ExternalInput")
    bias1 = nc.dram_tensor("bias1", [P, CB], F32, kind="ExternalInput")
    bias2 = nc.dram_tensor("bias2", [P, CB], F32, kind="ExternalInput")
    invl1 = nc.dram_tensor("invl1", [P, NSLOTS], F32, kind="ExternalInput")
    invl2 = nc.dram_tensor("invl2", [P, NSLOTS], F32, kind="ExternalInput")
    bfe = nc.dram_tensor("bfe", [P, NSLOTS], F32, kind="ExternalInput")
    out = nc.dram_tensor("out", [P, NSLOTS], F32, kind="ExternalOutput")

    RELU = mybir.ActivationFunctionType.Relu
    ADD = mybir.AluOpType.add
    MUL = mybir.AluOpType.mult
    MAX = mybir.AluOpType.max

    with tile.TileContext(nc) as tc:
        with (
            tc.tile_pool(name="consts", bufs=1) as consts,
            tc.tile_pool(name="h0p", bufs=3) as h0p,
            tc.tile_pool(name="h1p", bufs=3) as h1p,
            tc.tile_pool(name="mkp", bufs=4) as mkp,
            tc.tile_pool(name="scp", bufs=4) as scp,
            tc.tile_pool(name="psp", bufs=7, space="PSUM") as psp,
            tc.tile_pool(name="psv", bufs=1, space="PSUM") as psv,
        ):
            w1s = consts.tile([P, 2, 2, CB, P], F8)
            w14s = consts.tile([P, CB, P], F8)
            w2s = consts.tile([P, KW, CB, 2, P], F8)
            wfs = consts.tile([P, CB, P], BF)
            wcs = consts.tile([P, CB, P], BF)
            b1s = consts.tile([P, CB], F32)
            b2s = consts.tile([P, CB], F32)
            invl1s = consts.tile([P, NSLOTS], F32)
            invl2s = consts.tile([P, NSLOTS], F32)
            bfes = consts.tile([P, NSLOTS], F32)
            zcol = consts.tile([P, 1], F32)
            rs1 = consts.tile([P, NSLOTS, CB, NCH], F32)
            rs2 = consts.tile([P, NSLOTS, CB, NCH], F32)
            rs1_red = consts.tile([P, NSLOTS, CB], F32)
            rs2_red = consts.tile([P, NSLOTS, CB], F32)
            rs1b = consts.tile([P, NSLOTS, CB], BF)
            rs2b = consts.tile([P, NSLOTS, CB], BF)
            out_sb = consts.tile([P, NSLOTS], F32)

            h0_t = [None] * NSLOTS
            h1_t = [None] * NSLOTS
            mk_t = [None] * NSLOTS

            def emit_load(j, split_first=False):
                L1, L2, c0 = slot_cfg[j]
                if L1 == 0:
                    return
                h0 = h0p.tile([P, 2, H0W], F8, tag="h0")
                h1 = h1p.tile([P, CB, H1W], F8, tag="h1")
                h0_t[j], h1_t[j] = h0, h1
                w = min(L1 + 2, S)
                if split_first:
                    # first chunk (+halo) on the scalar queue, remaining
                    # chunks as separate pieces on sync — each matmul chunk
                    # only waits for its own piece.
                    w0 = min(CH + 6, w)
                    nc.scalar.dma_start(h0[:, 0, 2 : 2 + w0], xT[j, :, 0:w0])
                    nc.scalar.dma_start(h0[:, 1, 1 : 1 + w0], xT[j, :, 0:w0])
                    a = w0
                    while a < w:
                        b_ = min(a + CH, w)
                        nc.sync.dma_start(h0[:, 0, 2 + a : 2 + b_], xT[j, :, a:b_])
                        nc.sync.dma_start(h0[:, 1, 1 + a : 1 + b_], xT[j, :, a:b_])
                        a = b_
                else:
                    nc.sync.dma_start(h0[:, 0, 2 : 2 + w], xT[j, :, 0:w])
                    nc.sync.dma_start(h0[:, 1, 1 : 1 + w], xT[j, :, 0:w])
                nc.vector.memset(h0[:, 0, 0:2], 0.0)
                nc.vector.memset(h0[:, 1, 0:1], 0.0)
                if 2 + w < L1 + 4:
                    nc.vector.memset(h0[:, 0, 2 + w : L1 + 4], 0.0)
                if 1 + w < L1 + 2:
                    nc.vector.memset(h0[:, 1, 1 + w : L1 + 2], 0.0)
                for cb in range(CB):
                    nc.vector.memset(h1[:, cb, 0:2], 0.0)
                    nc.vector.memset(h1[:, cb, 2 + L1 : 4 + L1], 0.0)

            def emit_mask(j):
                L1, L2, c0 = slot_cfg[j]
                wm = L2 - c0 * CH
                if L1 > 0 and wm > 0:
                    mk = mkp.tile([P, S], BF, tag="mk")
                    mk_t[j] = mk
                    src = msk[j, c0 * CH : c0 * CH + wm]
                    bsrc = bass.AP(
                        tensor=src.tensor, offset=src.offset,
                        ap=[[0, P]] + list(src.ap),
                    )
                    nc.gpsimd.dma_start(mk[:, 0:wm], bsrc)

            def emit_conv1(j):
                L1, L2, c0 = slot_cfg[j]
                h0, h1 = h0_t[j], h1_t[j]
                for c, wc in enumerate(_chunks(L1)):
                    cs = c * CH
                    for cb in range(CB):
                        ps = psp.tile([P, CH], F32, tag="ps")
                        nc.tensor.matmul(
                            ps[:, 0:wc], w1s[:, 0, :, cb, :],
                            h0[:, :, cs : cs + wc],
                            start=True, stop=False, perf_mode=DR,
                        )
                        nc.tensor.matmul(
                            ps[:, 0:wc], w1s[:, 1, :, cb, :],
                            h0[:, :, cs + 2 : cs + 2 + wc],
                            start=False, stop=False, perf_mode=DR,
                        )
                        nc.tensor.matmul(
                            ps[:, 0:wc], w14s[:, cb, :],
                            h0[:, 0, cs + 4 : cs + 4 + wc],
                            start=False, stop=True,
                        )
                        # fused epilogue on DVE: h1 = max(psum + 16*b1, 0)
                        # with rowsum accumulated on every chunk — the fp8
                        # correction only needs a statistical mean of h1, so
                        # the unmasked sum over [0, L1) works (validated);
                        # the host folds 1/(WS*L1) instead of 1/(WS*len).
                        nc.vector.scalar_tensor_tensor(
                            h1[:, cb, 2 + cs : 2 + cs + wc],
                            ps[:, 0:wc], b1s[:, cb : cb + 1],
                            zcol.to_broadcast((P, wc)),
                            ADD, MAX, accum_out=rs1[:, j, cb, c : c + 1],
                        )

            def emit_conv2(j, full_last=False, eng=None):
                L1, L2, c0 = slot_cfg[j]
                h1, mk = h1_t[j], mk_t[j]
                order = list(enumerate(_chunks(L2)))
                if full_last:
                    # masked chunks (longer epilogue chains) first, fully
                    # accumulated chunks (single fused ACT) last, so the
                    # kernel tail only waits on the short chain.
                    order = [cw for cw in order if cw[0] >= c0] + \
                            [cw for cw in order if cw[0] < c0]
                for c, wc in order:
                    cs = c * CH
                    for cb in range(CB):
                        ps = psp.tile([P, CH], F32, tag="ps")
                        for k in range(KW):
                            nc.tensor.matmul(
                                ps[:, 0:wc],
                                w2s[:, k, cb, :, :],
                                h1[:, :, cs + k : cs + k + wc],
                                start=(k == 0), stop=(k == KW - 1),
                                perf_mode=DR,
                            )
                        h2 = scp.tile([P, CH], BF, tag="h2")
                        if c < c0:
                            # unmasked for every core in the group:
                            # ReLU + bias + rowsum fused on ScalarE
                            nc.scalar.activation(
                                h2[:, 0:wc], ps[:, 0:wc], RELU,
                                bias=b2s[:, cb : cb + 1],
                                accum_out=rs2[:, j, cb, c : c + 1],
                            )
                        else:
                            nc.scalar.activation(
                                h2[:, 0:wc], ps[:, 0:wc], RELU,
                                bias=b2s[:, cb : cb + 1],
                            )
                            sc = scp.tile([P, CH], BF, tag="sc")
                            eng.scalar_tensor_tensor(
                                sc[:, 0:wc], h2[:, 0:wc], 1.0,
                                mk[:, cs - c0 * CH : cs - c0 * CH + wc],
                                MUL, MUL,
                                accum_out=rs2[:, j, cb, c : c + 1],
                            )

            def emit_slot_reduce(j):
                L1, L2, c0 = slot_cfg[j]
                for rs, red, nc_ in (
                    (rs2, rs2_red, len(_chunks(L2))),
                    (rs1, rs1_red, len(_chunks(L1))),
                ):
                    for cb in range(CB):
                        if nc_ == 0:
                            nc.vector.memset(red[:, j, cb : cb + 1], 0.0)
                        elif nc_ == 1:
                            nc.vector.tensor_copy(
                                red[:, j, cb : cb + 1], rs[:, j, cb, 0:1]
                            )
                        else:
                            nc.vector.tensor_reduce(
                                red[:, j, cb : cb + 1],
                                rs[:, j, cb, 0:nc_],
                                axis=mybir.AxisListType.X, op=ADD,
                            )
                # fold 1/(scale*len) so the tail is only matvec + bias;
                # bf16 output feeds the single-pass bf16 matvec
                nc.vector.tensor_tensor(
                    rs2b[:, j, :], rs2_red[:, j, :],
                    invl2s[:, j : j + 1].to_broadcast((P, CB)), MUL,
                )
                nc.vector.tensor_tensor(
                    rs1b[:, j, :], rs1_red[:, j, :],
                    invl1s[:, j : j + 1].to_broadcast((P, CB)), MUL,
                )

            # ---- emission order ----
            # PE warmup: the first data DMAs cannot complete before ~12us
            # (engine preamble + descriptor processing); dummy matmuls keep
            # the PE busy so the HAM clock gate is released (8/8 = 2.4 GHz)
            # by the time real matmuls issue.
            warm_w = scp.tile([P, CH], BF, tag="warm")
            warm_ps = psp.tile([P, CH], F32, tag="ps")
            nc.gpsimd.memset(warm_w, 0.0)
            nc.vector.memset(zcol, 0.0)
            for _ in range(9):
                nc.tensor.matmul(warm_ps, warm_w[:, 0:P], warm_w,
                                 start=True, stop=True)

            # conv1 weights on sync (ahead of x pieces); w2 on gpsimd SWDGE;
            # slot 0's first x chunk on the scalar queue (behind only the
            # ACT table load).
            nc.sync.dma_start(w1s, w1t[:])
            nc.sync.dma_start(w14s, w14t[:])
            emit_load(0, split_first=True)
            nc.gpsimd.dma_start(w2s, w2t[:])
            nc.scalar.dma_start(b1s, bias1[:])
            nc.scalar.dma_start(b2s, bias2[:])
            nc.scalar.dma_start(invl1s, invl1[:])
            nc.scalar.dma_start(invl2s, invl2[:])
            emit_load(1)
            emit_mask(0)
            emit_mask(1)
            emit_mask(2)
            emit_mask(3)
            emit_conv1(0)
            emit_load(2)
            emit_conv1(1)
            emit_conv2(0, eng=nc.vector)
            emit_slot_reduce(0)
            emit_load(3)
            emit_conv1(2)
            emit_conv2(1, eng=nc.vector)
            emit_slot_reduce(1)
            nc.scalar.dma_start(wfs, wft[:])
            nc.scalar.dma_start(wcs, wct[:])
            nc.scalar.dma_start(bfes, bfe[:])
            emit_conv1(3)
            emit_conv2(3, eng=nc.vector)
            emit_slot_reduce(3)
            emit_conv2(2, full_last=True, eng=nc.vector)
            emit_slot_reduce(2)
            # single bf16 matvec for all samples: 1x1-conv + fp8-correction
            pooled = psv.tile([P, NSLOTS], F32)
            ops = [(wfs, rs2b), (wcs, rs1b)]
            i = 0
            for w_, r_ in ops:
                for cb in range(CB):
                    nc.tensor.matmul(
                        pooled[:, 0:NSLOTS],
                        w_[:, cb, :],
                        r_[:, :, cb],
                        start=(i == 0),
                        stop=(i == 2 * CB - 1),
                    )
                    i += 1
            nc.vector.tensor_tensor(out_sb, pooled, bfes, ADD)
            nc.sync.dma_start(out[:], out_sb)

    nc.compile()
    return nc


def _prep(inputs):
    """Host-side: BN folding, fp8 weight packing, length-sorted slots."""
    x = np.ascontiguousarray(np.asarray(inputs["x"], dtype=np.float32))
    spi = np.asarray(inputs["start_padding_indices"]).astype(np.int64).reshape(B)
    W1 = np.asarray(inputs["W1"], np.float32)
    b1 = np.asarray(inputs["b1"], np.float32)
    g1 = np.asarray(inputs["g1"], np.float32)
    be1 = np.asarray(inputs["be1"], np.float32)
    m1 = np.asarray(inputs["m1"], np.float32)
    v1 = np.asarray(inputs["v1"], np.float32)
    W2 = np.asarray(inputs["W2"], np.float32)
    b2 = np.asarray(inputs["b2"], np.float32)
    g2 = np.asarray(inputs["g2"], np.float32)
    be2 = np.asarray(inputs["be2"], np.float32)
    m2 = np.asarray(inputs["m2"], np.float32)
    v2 = np.asarray(inputs["v2"], np.float32)
    Wf = np.asarray(inputs["Wf"], np.float32)
    bf = np.asarray(inputs["bf"], np.float32)

    lens = np.where(spi == -1, S, spi)
    lens = np.clip(lens, 0, S).astype(np.int64)

    order = np.argsort(-lens, kind="stable")
    assign = order.reshape(NSLOTS, NCORES)  # [slot, core] -> sample idx

    slot_cfg = []
    for j in range(NSLOTS):
        lj = lens[assign[j]]
        lmax, lmin = int(lj.max()), int(lj.min())
        if lmax == 0:
            slot_cfg.append((0, 0, 0))
            continue
        L2 = min(math.ceil(lmax / GR) * GR, S)
        L1 = min(math.ceil(min(lmax + 2, S) / GR) * GR, S)
        c0 = min(lmin // CH, len(_chunks(L2)))
        slot_cfg.append((L1, L2, c0))
    slot_cfg = tuple(slot_cfg)

    # fold BN into conv weights/biases
    s1 = g1 / np.sqrt(v1 + EPS)
    W1f = W1 * s1[:, None, None]
    b1f = (b1 - m1) * s1 + be1
    s2 = g2 / np.sqrt(v2 + EPS)
    W2f = W2 * s2[:, None, None]
    b2f = (b2 - m2) * s2 + be2

    # fp8 weights, scaled by WS (scale folded downstream)
    W1q = np.clip(W1f * WS, -240, 240).astype(NP8)   # [C, D, K]
    W2q = np.clip(W2f * WS, -240, 240).astype(NP8)   # [C, C, K]

    # conv1 DoubleRow packs: [d, pair, i, cb, co] for taps 0..3, tap 4 alone
    a1 = np.asarray(W1q).reshape(CB, P, D, KW)                 # [cb, co, d, k]
    w1t = np.ascontiguousarray(
        a1[:, :, :, 0:4].reshape(CB, P, D, 2, 2).transpose(2, 3, 4, 0, 1)
    )                                                          # [d, p, i, cb, co]
    w14t = np.ascontiguousarray(a1[:, :, :, 4].transpose(2, 0, 1))  # [d, cb, co]
    # conv2 DoubleRow pack: pair = input channel block
    a2 = np.asarray(W2q).reshape(CB, P, 2, P, KW)              # [cob, co, i, ci, k]
    w2t = np.ascontiguousarray(a2.transpose(3, 4, 0, 2, 1))    # [ci, k, cob, i, co]

    # fp8 W2 error correction: Wc = 0.5 * Wf @ sum_k(W2f - deq(W2q))
    dW2eff = (W2f - np.asarray(W2q).astype(np.float32) / WS).sum(axis=2)  # [co, ci]
    Wc = 0.5 * (Wf[:, :, 0] @ dW2eff)                          # [d, ci]

    wft = np.ascontiguousarray(
        Wf[:, :, 0].reshape(D, CB, P).transpose(2, 1, 0)
    ).astype(BF16)  # [ci, cb, d]
    wct = np.ascontiguousarray(
        Wc.reshape(D, CB, P).transpose(2, 1, 0)
    ).astype(BF16)
    bias1 = np.ascontiguousarray((WS * b1f).reshape(CB, P).T).astype(np.float32)
    bias2 = np.ascontiguousarray((WS * WS * b2f).reshape(CB, P).T).astype(np.float32)

    t_idx = np.arange(S)
    in_maps = []
    for i in range(NCORES):
        xT_i = np.empty((NSLOTS, P, S), dtype=NP8)
        msk_i = np.zeros((NSLOTS, S), dtype=BF16)
        invl1_i = np.empty((P, NSLOTS), dtype=np.float32)
        invl2_i = np.empty((P, NSLOTS), dtype=np.float32)
        bfe_i = np.empty((P, NSLOTS), dtype=np.float32)
        for j in range(NSLOTS):
            b_idx = int(assign[j, i])
            L = int(lens[b_idx])
            L1j = slot_cfg[j][0]
            xT_i[j] = np.clip(x[b_idx].T, -240, 240).astype(NP8)
            msk_i[j] = (t_idx < L).astype(BF16)
            # rs1 is the UNMASKED h1 sum over [0, L1): statistical mean for
            # the fp8 correction; zeroed entirely for empty samples
            invl1_i[:, j] = 1.0 / (WS * L1j) if L > 0 else 0.0
            invl2_i[:, j] = 1.0 / (WS * WS * max(L, 1))
            bfe_i[:, j] = bf * (1.0 if L > 0 else 0.0)
        in_maps.append({
            "xT": xT_i, "msk": msk_i,
            "w1t": w1t, "w14t": w14t, "w2t": w2t,
            "wft": wft, "wct": wct,
            "bias1": bias1, "bias2": bias2,
            "invl1": invl1_i, "invl2": invl2_i, "bfe": bfe_i,
        })
    return slot_cfg, assign, in_maps


def kernel(**inputs) -> np.ndarray:
    global LAST_RESULTS
    import os

    slot_cfg, assign, in_maps = _prep(inputs)
    nc = _BUILD_CACHE.get(slot_cfg)
    if nc is None:
        nc = _build(slot_cfg)
        _BUILD_CACHE[slot_cfg] = nc

    trace = TRACE or bool(os.environ.get("BASS_TRACE"))
    if trace:
        try:
            import antenv.axon_hooks  # noqa: F401  (absent in some containers)
        except ImportError:
            trace = False
    res = run_bass_kernel_spmd(
        nc, in_maps, core_ids=list(range(NCORES)), trace=trace,
    )
    LAST_RESULTS = res

    pooled = np.zeros((B, D), dtype=np.float32)
    for i in range(NCORES):
        out_i = np.asarray(res.results[i]["out"], dtype=np.float32)  # [P, NSLOTS]
        for j in range(NSLOTS):
            pooled[int(assign[j, i])] = out_i[:, j]
    return pooled


# revision 20
# speedup vs baseline: 1.4820x; 1.0121x over previous
"""Trainium2 Bass kernel for nn_CNNBackbone: conv1d(D->C,K=5) + BN + ReLU,
conv1d(C->C,K=5) + BN + ReLU, conv1d(C->D,1x1), masked mean over ragged lengths.

Strategy (v2: fp8 DoubleRow)
----------------------------
Data-parallel over batch: 32 samples -> 8 cores x 4 sample-slots, sorted by
length so each slot's group of 8 has near-uniform length; per-slot loop bounds
are compile-time constants from the group max (SPMD: one program, 8 cores).

Numerics: both convs run in fp8-e4m3 with MatmulPerfMode.DoubleRow (2 fp8
contraction elements per PE cell per cycle -> ~1.8x tensor throughput).
 - conv2 pairs the two 128-channel input blocks: h1 is stored [P, 2, T] fp8,
   which is exactly the DoubleRow rhs layout.
 - conv1 pairs adjacent taps; x is DMA'd twice into a [P, 2, W] tile with a
   one-column shift so tap pairs (0,1) and (2,3) are DoubleRow matmuls and
   tap 4 is a normal fp8 matmul.
 - weights are scaled by 16 (keeps e4m3 out of subnormals); the scale is
   folded downstream (h1 stored as 16*h1, rowsums folded via 1/(16L),
   1/(256L) host-side constants), so conv epilogues are a SINGLE fused
   scalar_tensor_tensor op: out = max(psum + 16*b, 0) with accum_out rowsum.
 - fp8 W2 quantization error is weight-correlated and survives pooling
   (~1.9e-2 rel); corrected by pooled += 0.5 * dW2eff @ pooled(h1), folded
   into one extra per-sample matvec with host-precomputed Wc = 0.5*Wf@dW2eff.
   Net rel err ~8e-3 (numpy-validated).

Engine split: PE matmuls; conv1 epilogue on DVE (fused stt), conv2 epilogue
on ScalarE ACT (bias+relu+accum); partial-length chunks use one fused
mask-multiply-accumulate stt on DVE. The final 1x1 conv commutes with the
masked mean so it is a tiny per-sample fp32 matvec.
"""

import math

import numpy as np
import ml_dtypes

import concourse.bass as bass
import concourse.mybir as mybir
import concourse.tile as tile
from concourse import bacc
from concourse.bass_utils import run_bass_kernel_spmd

B, S, D, C, KW = 32, 2048, 128, 256, 5
P = 128
CH = 512            # full chunk (matmul free dim / PSUM bank)
GR = 16             # tail-chunk width granularity
NCORES = 8
NSLOTS = B // NCORES
CB = C // P         # channel blocks of 128
EPS = 1e-5
WS = 16.0           # fp8 weight scale (power of 2)
H0W = S + 16        # x pair-buffer lane width (16-aligned)
H1W = S + 16        # h1 lane width (16-aligned)
NCH = S // CH + 1   # max chunks per slot
BF16 = ml_dtypes.bfloat16
NP8 = ml_dtypes.float8_e4m3
F32 = mybir.dt.float32
BF = mybir.dt.bfloat16
F8 = mybir.dt.float8e4
DR = mybir.MatmulPerfMode.DoubleRow

_BUILD_CACHE: dict = {}
LAST_RESULTS = None  # BassKernelResults of the most recent run (for test harness)
TRACE = False        # set True (or env BASS_TRACE=1) to capture a profile


def _chunks(total):
    """Split `total` columns into 512-wide chunks plus a short tail."""
    ws = [CH] * (total // CH)
    if total % CH:
        ws.append(total % CH)
    return ws


def _build(slot_cfg):
    """Build + compile the SPMD Bass program.

    slot_cfg[j] = (L1, L2, c0): conv1/conv2 computed column counts (multiples
    of GR) and the count of fully-unmasked 512-chunks for slot j's group.
    """
    nc = bacc.Bacc(None, target_bir_lowering=False, debug=False)

    xT = nc.dram_tensor("xT", [NSLOTS, P, S], F8, kind="ExternalInput")
    msk = nc.dram_tensor("msk", [NSLOTS, S], BF, kind="ExternalInput")
    w1t = nc.dram_tensor("w1t", [P, 2, 2, CB, P], F8, kind="ExternalInput")
    w14t = nc.dram_tensor("w14t", [P, CB, P], F8, kind="ExternalInput")
    w2t = nc.dram_tensor("w2t", [P, KW, CB, 2, P], F8, kind="ExternalInput")
    wft = nc.dram_tensor("wft", [P, CB, P], BF, kind="ExternalInput")
    wct = nc.dram_tensor("wct", [P, CB, P], BF, kind="ExternalInput")
    bias1 = nc.dram_tensor("bias1", [P, CB], F32, kind="ExternalInput")
    bias2 = nc.dram_tensor("bias2", [P, CB], F32, kind="ExternalInput")
    invl1 = nc.dram_tensor("invl1", [P, NSLOTS], F32, kind="ExternalInput")
    invl2 = nc.dram_tensor("invl2", [P, NSLOTS], F32, kind="ExternalInput")
    bfe = nc.dram_tensor("bfe", [P, NSLOTS], F32, kind="ExternalInput")
    out = nc.dram_tensor("out", [P, NSLOTS], F32, kind="ExternalOutput")

    RELU = mybir.ActivationFunctionType.Relu
    ADD = mybir.AluOpType.add
    MUL = mybir.AluOpType.mult
    MAX = mybir.AluOpType.max

    with tile.TileContext(nc) as tc:
        with (
            tc.tile_pool(name="consts", bufs=1) as consts,
            tc.tile_pool(name="h0p", bufs=3) as h0p,
            tc.tile_pool(name="h1p", bufs=3) as h1p,
            tc.tile_pool(name="mkp", bufs=4) as mkp,
            tc.tile_pool(name="scp", bufs=4) as scp,
            tc.tile_pool(name="psp", bufs=7, space="PSUM") as psp,
            tc.tile_pool(name="psv", bufs=1, space="PSUM") as psv,
        ):
            w1s = consts.tile([P, 2, 2, CB, P], F8)
            w14s = consts.tile([P, CB, P], F8)
            w2s = consts.tile([P, KW, CB, 2, P], F8)
            wfs = consts.tile([P, CB, P], BF)
            wcs = consts.tile([P, CB, P], BF)
            b1s = consts.tile([P, CB], F32)
            b2s = consts.tile([P, CB], F32)
            invl1s = consts.tile([P, NSLOTS], F32)
            invl2s = consts.tile([P, NSLOTS], F32)
            bfes = consts.tile([P, NSLOTS], F32)
            zcol = consts.tile([P, 1], F32)
            rs1 = consts.tile([P, NSLOTS, CB, NCH], F32)
            rs2 = consts.tile([P, NSLOTS, CB, NCH], F32)
            rs1_red = consts.tile([P, NSLOTS, CB], F32)
            rs2_red = consts.tile([P, NSLOTS, CB], F32)
            rs1b = consts.tile([P, NSLOTS, CB], BF)
            rs2b = consts.tile([P, NSLOTS, CB], BF)
            out_sb = consts.tile([P, NSLOTS], F32)

            h0_t = [None] * NSLOTS
            h1_t = [None] * NSLOTS
            mk_t = [None] * NSLOTS

            def emit_load(j, split_first=False):
                L1, L2, c0 = slot_cfg[j]
                if L1 == 0:
                    return
                h0 = h0p.tile([P, 2, H0W], F8, tag="h0")
                h1 = h1p.tile([P, CB, H1W], F8, tag="h1")
                h0_t[j], h1_t[j] = h0, h1
                w = min(L1 + 2, S)
                if split_first:
                    # first chunk (+halo) lanes on the sync and gpsimd queues
                    # (parallel descriptor processing, no ACT-table ahead);
                    # remaining chunks as separate pieces on sync — each
                    # matmul chunk only waits for its own piece.
                    w0 = min(CH + 6, w)
                    nc.sync.dma_start(h0[:, 0, 2 : 2 + w0], xT[j, :, 0:w0])
                    nc.gpsimd.dma_start(h0[:, 1, 1 : 1 + w0], xT[j, :, 0:w0])
                    a = w0
                    while a < w:
                        b_ = min(a + CH, w)
                        nc.sync.dma_start(h0[:, 0, 2 + a : 2 + b_], xT[j, :, a:b_])
                        nc.sync.dma_start(h0[:, 1, 1 + a : 1 + b_], xT[j, :, a:b_])
                        a = b_
                else:
                    nc.sync.dma_start(h0[:, 0, 2 : 2 + w], xT[j, :, 0:w])
                    nc.sync.dma_start(h0[:, 1, 1 : 1 + w], xT[j, :, 0:w])
                nc.vector.memset(h0[:, 0, 0:2], 0.0)
                nc.vector.memset(h0[:, 1, 0:1], 0.0)
                if 2 + w < L1 + 4:
                    nc.vector.memset(h0[:, 0, 2 + w : L1 + 4], 0.0)
                if 1 + w < L1 + 2:
                    nc.vector.memset(h0[:, 1, 1 + w : L1 + 2], 0.0)
                for cb in range(CB):
                    nc.vector.memset(h1[:, cb, 0:2], 0.0)
                    nc.vector.memset(h1[:, cb, 2 + L1 : 4 + L1], 0.0)

            def emit_mask(j):
                L1, L2, c0 = slot_cfg[j]
                wm = L2 - c0 * CH
                if L1 > 0 and wm > 0:
                    mk = mkp.tile([P, S], BF, tag="mk")
                    mk_t[j] = mk
                    src = msk[j, c0 * CH : c0 * CH + wm]
                    bsrc = bass.AP(
                        tensor=src.tensor, offset=src.offset,
                        ap=[[0, P]] + list(src.ap),
                    )
                    nc.gpsimd.dma_start(mk[:, 0:wm], bsrc)

            def emit_conv1(j):
                L1, L2, c0 = slot_cfg[j]
                h0, h1 = h0_t[j], h1_t[j]
                for c, wc in enumerate(_chunks(L1)):
                    cs = c * CH
                    for cb in range(CB):
                        ps = psp.tile([P, CH], F32, tag="ps")
                        nc.tensor.matmul(
                            ps[:, 0:wc], w1s[:, 0, :, cb, :],
                            h0[:, :, cs : cs + wc],
                            start=True, stop=False, perf_mode=DR,
                        )
                        nc.tensor.matmul(
                            ps[:, 0:wc], w1s[:, 1, :, cb, :],
                            h0[:, :, cs + 2 : cs + 2 + wc],
                            start=False, stop=False, perf_mode=DR,
                        )
                        nc.tensor.matmul(
                            ps[:, 0:wc], w14s[:, cb, :],
                            h0[:, 0, cs + 4 : cs + 4 + wc],
                            start=False, stop=True,
                        )
                        # fused epilogue on DVE: h1 = max(psum + 16*b1, 0)
                        # with rowsum accumulated on every chunk — the fp8
                        # correction only needs a statistical mean of h1, so
                        # the unmasked sum over [0, L1) works (validated);
                        # the host folds 1/(WS*L1) instead of 1/(WS*len).
                        nc.vector.scalar_tensor_tensor(
                            h1[:, cb, 2 + cs : 2 + cs + wc],
                            ps[:, 0:wc], b1s[:, cb : cb + 1],
                            zcol.to_broadcast((P, wc)),
                            ADD, MAX, accum_out=rs1[:, j, cb, c : c + 1],
                        )

            def emit_conv2(j, full_last=False, eng=None):
                L1, L2, c0 = slot_cfg[j]
                h1, mk = h1_t[j], mk_t[j]
                order = list(enumerate(_chunks(L2)))
                if full_last:
                    # masked chunks (longer epilogue chains) first, fully
                    # accumulated chunks (single fused ACT) last, so the
                    # kernel tail only waits on the short chain.
                    order = [cw for cw in order if cw[0] >= c0] + \
                            [cw for cw in order if cw[0] < c0]
                for c, wc in order:
                    cs = c * CH
                    for cb in range(CB):
                        ps = psp.tile([P, CH], F32, tag="ps")
                        for k in range(KW):
                            nc.tensor.matmul(
                                ps[:, 0:wc],
                                w2s[:, k, cb, :, :],
                                h1[:, :, cs + k : cs + k + wc],
                                start=(k == 0), stop=(k == KW - 1),
                                perf_mode=DR,
                            )
                        h2 = scp.tile([P, CH], BF, tag="h2")
                        if c < c0:
                            # unmasked for every core in the group:
                            # ReLU + bias + rowsum fused on ScalarE
                            nc.scalar.activation(
                                h2[:, 0:wc], ps[:, 0:wc], RELU,
                                bias=b2s[:, cb : cb + 1],
                                accum_out=rs2[:, j, cb, c : c + 1],
                            )
                        else:
                            nc.scalar.activation(
                                h2[:, 0:wc], ps[:, 0:wc], RELU,
                                bias=b2s[:, cb : cb + 1],
                            )
                            sc = scp.tile([P, CH], BF, tag="sc")
                            eng.scalar_tensor_tensor(
                                sc[:, 0:wc], h2[:, 0:wc], 1.0,
                                mk[:, cs - c0 * CH : cs - c0 * CH + wc],
                                MUL, MUL,
                                accum_out=rs2[:, j, cb, c : c + 1],
                            )

            def emit_slot_reduce(j):
                L1, L2, c0 = slot_cfg[j]
                for rs, red, nc_ in (
                    (rs2, rs2_red, len(_chunks(L2))),
                    (rs1, rs1_red, len(_chunks(L1))),
                ):
                    for cb in range(CB):
                        if nc_ == 0:
                            nc.vector.memset(red[:, j, cb : cb + 1], 0.0)
                        elif nc_ == 1:
                            nc.vector.tensor_copy(
                                red[:, j, cb : cb + 1], rs[:, j, cb, 0:1]
                            )
                        else:
                            nc.vector.tensor_reduce(
                                red[:, j, cb : cb + 1],
                                rs[:, j, cb, 0:nc_],
                                axis=mybir.AxisListType.X, op=ADD,
                            )
                # fold 1/(scale*len) so the tail is only matvec + bias;
                # bf16 output feeds the single-pass bf16 matvec
                nc.vector.tensor_tensor(
                    rs2b[:, j, :], rs2_red[:, j, :],
                    invl2s[:, j : j + 1].to_broadcast((P, CB)), MUL,
                )
                nc.vector.tensor_tensor(
                    rs1b[:, j, :], rs1_red[:, j, :],
                    invl1s[:, j : j + 1].to_broadcast((P, CB)), MUL,
                )

            # ---- emission order ----
            # PE warmup: the first data DMAs cannot complete before ~12us
            # (engine preamble + descriptor processing); dummy matmuls keep
            # the PE busy so the HAM clock gate is released (8/8 = 2.4 GHz)
            # by the time real matmuls issue.
            warm_w = scp.tile([P, CH], BF, tag="warm")
            warm_ps = psp.tile([P, CH], F32, tag="ps")
            nc.gpsimd.memset(warm_w, 0.0)
            for _ in range(8):
                nc.tensor.matmul(warm_ps, warm_w[:, 0:P], warm_w,
                                 start=True, stop=True)

            # slot 0's first x chunk lanes go first on the sync/gpsimd
            # queues; all weights on gpsimd SWDGE; biases on the scalar
            # queue (behind the ACT table load, needed late).
            nc.gpsimd.dma_start(w1s, w1t[:])
            emit_load(0, split_first=True)
            nc.vector.memset(zcol, 0.0)
            nc.gpsimd.dma_start(w14s, w14t[:])
            nc.gpsimd.dma_start(w2s, w2t[:])
            nc.scalar.dma_start(b1s, bias1[:])
            nc.scalar.dma_start(b2s, bias2[:])
            nc.scalar.dma_start(invl1s, invl1[:])
            nc.scalar.dma_start(invl2s, invl2[:])
            emit_load(1)
            emit_mask(0)
            emit_mask(1)
            emit_mask(2)
            emit_mask(3)
            emit_conv1(0)
            emit_load(2)
            emit_conv1(1)
            emit_conv2(0, eng=nc.vector)
            emit_slot_reduce(0)
            emit_load(3)
            emit_conv1(2)
            emit_conv2(1, eng=nc.vector)
            emit_slot_reduce(1)
            nc.scalar.dma_start(wfs, wft[:])
            nc.scalar.dma_start(wcs, wct[:])
            nc.scalar.dma_start(bfes, bfe[:])
            emit_conv1(3)
            emit_conv2(3, eng=nc.vector)
            emit_slot_reduce(3)
            emit_conv2(2, full_last=True, eng=nc.vector)
            emit_slot_reduce(2)
            # single bf16 matvec for all samples: 1x1-conv + fp8-correction
            pooled = psv.tile([P, NSLOTS], F32)
            ops = [(wfs, rs2b), (wcs, rs1b)]
            i = 0
            for w_, r_ in ops:
                for cb in range(CB):
                    nc.tensor.matmul(
                        pooled[:, 0:NSLOTS],
                        w_[:, cb, :],
                        r_[:, :, cb],
                        start=(i == 0),
                        stop=(i == 2 * CB - 1),
                    )
                    i += 1
            nc.vector.tensor_tensor(out_sb, pooled, bfes, ADD)
            nc.sync.dma_start(out[:], out_sb)

    nc.compile()
    return nc


def _prep(inputs):
    """Host-side: BN folding, fp8 weight packing, length-sorted slots."""
    x = np.ascontiguousarray(np.asarray(inputs["x"], dtype=np.float32))
    spi = np.asarray(inputs["start_padding_indices"]).astype(np.int64).reshape(B)
    W1 = np.asarray(inputs["W1"], np.float32)
    b1 = np.asarray(inputs["b1"], np.float32)
    g1 = np.asarray(inputs["g1"], np.float32)
    be1 = np.asarray(inputs["be1"], np.float32)
    m1 = np.asarray(inputs["m1"], np.float32)
    v1 = np.asarray(inputs["v1"], np.float32)
    W2 = np.asarray(inputs["W2"], np.float32)
    b2 = np.asarray(inputs["b2"], np.float32)
    g2 = np.asarray(inputs["g2"], np.float32)
    be2 = np.asarray(inputs["be2"], np.float32)
    m2 = np.asarray(inputs["m2"], np.float32)
    v2 = np.asarray(inputs["v2"], np.float32)
    Wf = np.asarray(inputs["Wf"], np.float32)
    bf = np.asarray(inputs["bf"], np.float32)

    lens = np.where(spi == -1, S, spi)
    lens = np.clip(lens, 0, S).astype(np.int64)

    order = np.argsort(-lens, kind="stable")
    assign = order.reshape(NSLOTS, NCORES)  # [slot, core] -> sample idx

    slot_cfg = []
    for j in range(NSLOTS):
        lj = lens[assign[j]]
        lmax, lmin = int(lj.max()), int(lj.min())
        if lmax == 0:
            slot_cfg.append((0, 0, 0))
            continue
        L2 = min(math.ceil(lmax / GR) * GR, S)
        L1 = min(math.ceil(min(lmax + 2, S) / GR) * GR, S)
        c0 = min(lmin // CH, len(_chunks(L2)))
        slot_cfg.append((L1, L2, c0))
    slot_cfg = tuple(slot_cfg)

    # fold BN into conv weights/biases
    s1 = g1 / np.sqrt(v1 + EPS)
    W1f = W1 * s1[:, None, None]
    b1f = (b1 - m1) * s1 + be1
    s2 = g2 / np.sqrt(v2 + EPS)
    W2f = W2 * s2[:, None, None]
    b2f = (b2 - m2) * s2 + be2

    # fp8 weights, scaled by WS (scale folded downstream)
    W1q = np.clip(W1f * WS, -240, 240).astype(NP8)   # [C, D, K]
    W2q = np.clip(W2f * WS, -240, 240).astype(NP8)   # [C, C, K]

    # conv1 DoubleRow packs: [d, pair, i, cb, co] for taps 0..3, tap 4 alone
    a1 = np.asarray(W1q).reshape(CB, P, D, KW)                 # [cb, co, d, k]
    w1t = np.ascontiguousarray(
        a1[:, :, :, 0:4].reshape(CB, P, D, 2, 2).transpose(2, 3, 4, 0, 1)
    )                                                          # [d, p, i, cb, co]
    w14t = np.ascontiguousarray(a1[:, :, :, 4].transpose(2, 0, 1))  # [d, cb, co]
    # conv2 DoubleRow pack: pair = input channel block
    a2 = np.asarray(W2q).reshape(CB, P, 2, P, KW)              # [cob, co, i, ci, k]
    w2t = np.ascontiguousarray(a2.transpose(3, 4, 0, 2, 1))    # [ci, k, cob, i, co]

    # fp8 W2 error correction: Wc = 0.5 * Wf @ sum_k(W2f - deq(W2q))
    dW2eff = (W2f - np.asarray(W2q).astype(np.float32) / WS).sum(axis=2)  # [co, ci]
    Wc = 0.5 * (Wf[:, :, 0] @ dW2eff)                          # [d, ci]

    wft = np.ascontiguousarray(
        Wf[:, :, 0].reshape(D, CB, P).transpose(2, 1, 0)
    ).astype(BF16)  # [ci, cb, d]
    wct = np.ascontiguousarray(
        Wc.reshape(D, CB, P).transpose(2, 1, 0)
    ).astype(BF16)
    bias1 = np.ascontiguousarray((WS * b1f).reshape(CB, P).T).astype(np.float32)
    bias2 = np.ascontiguousarray((WS * WS * b2f).reshape(CB, P).T).astype(np.float32)

    t_idx = np.arange(S)
    in_maps = []
    for i in range(NCORES):
        xT_i = np.empty((NSLOTS, P, S), dtype=NP8)
        msk_i = np.zeros((NSLOTS, S), dtype=BF16)
        invl1_i = np.empty((P, NSLOTS), dtype=np.float32)
        invl2_i = np.empty((P, NSLOTS), dtype=np.float32)
        bfe_i = np.empty((P, NSLOTS), dtype=np.float32)
        for j in range(NSLOTS):
            b_idx = int(assign[j, i])
            L = int(lens[b_idx])
            L1j = slot_cfg[j][0]
            xT_i[j] = np.clip(x[b_idx].T, -240, 240).astype(NP8)
            msk_i[j] = (t_idx < L).astype(BF16)
            # rs1 is the UNMASKED h1 sum over [0, L1): statistical mean for
            # the fp8 correction; zeroed entirely for empty samples
            invl1_i[:, j] = 1.0 / (WS * L1j) if L > 0 else 0.0
            invl2_i[:, j] = 1.0 / (WS * WS * max(L, 1))
            bfe_i[:, j] = bf * (1.0 if L > 0 else 0.0)
        in_maps.append({
            "xT": xT_i, "msk": msk_i,
            "w1t": w1t, "w14t": w14t, "w2t": w2t,
            "wft": wft, "wct": wct,
            "bias1": bias1, "bias2": bias2,
            "invl1": invl1_i, "invl2": invl2_i, "bfe": bfe_i,
        })
    return slot_cfg, assign, in_maps


def kernel(**inputs) -> np.ndarray:
    global LAST_RESULTS
    import os

    slot_cfg, assign, in_maps = _prep(inputs)
    nc = _BUILD_CACHE.get(slot_cfg)
    if nc is None:
        nc = _build(slot_cfg)
        _BUILD_CACHE[slot_cfg] = nc

    trace = TRACE or bool(os.environ.get("BASS_TRACE"))
    if trace:
        try:
            import antenv.axon_hooks  # noqa: F401  (absent in some containers)
        except ImportError:
            trace = False
    res = run_bass_kernel_spmd(
        nc, in_maps, core_ids=list(range(NCORES)), trace=trace,
    )
    LAST_RESULTS = res

    pooled = np.zeros((B, D), dtype=np.float32)
    for i in range(NCORES):
        out_i = np.asarray(res.results[i]["out"], dtype=np.float32)  # [P, NSLOTS]
        for j in range(NSLOTS):
            pooled[int(assign[j, i])] = out_i[:, j]
    return pooled
